# revision 6
# baseline (speedup 1.0000x reference)
"""Trainium2 Bass kernel for nn_BackMapLayer (internal-coords -> cartesian backmap).

Reformulation: the reference's sequential tail-rotation chain rebuild equals
  1) in-plane chain via cumsums (heading angles, bond vectors, positions)
  2) per-bond rotation quaternions g_i = (cos(d_i/2), -sin(d_i/2) * axis_i)
     with axis_i the *in-plane* unit bond and d_i = -(dihedral_i + pi)
  3) prefix products h_k = g_0 x ... x g_k (Hillis-Steele scan)
  4) final bonds = in-plane bonds rotated by h_{k-1}; positions = cumsum.
Data parallel over batch: 512 rows -> 8 cores x 64 rows. Each core also
computes the full-batch mean of distances itself (distances replicated).
Left chain lives on partitions 64..127, right chain on partitions 0..63;
the per-core 64-row shard is duplicated onto both partition halves.
"""
import os
from contextlib import ExitStack

import numpy as np

import concourse.bacc as bacc
import concourse.bass as bass
import concourse.mybir as mybir
import concourse.tile as tile
from concourse import bass_utils

F32 = mybir.dt.float32
PI = float(np.pi)
TWO_PI = float(2.0 * np.pi)

NB = 64              # batch rows per core
N_CORES = 8
B_FULL = NB * N_CORES
N_ATOMS = 512
N_DIST = 511
N_ANG = 510
N_DIH = 509
RIGHT_SPLIT = 254    # rotations in right chain
LEFT_SPLIT = 255     # rotations in left chain
NSCAN = 256          # padded quaternion-scan width

Alu = mybir.AluOpType
Act = mybir.ActivationFunctionType

R = slice(0, 64)     # right-chain partition rows
L = slice(64, 128)   # left-chain partition rows


def _emit_quat_mul(nc, pool, out, a, b, width, sparse_z=False, gpsimd_mults=2):
    """out = a (x) b elementwise quaternion product (Hamilton).

    out/a/b: 4-tuples of APs, each [P, width]. sparse_z: a_z == b_z == 0.
    gpsimd_mults: how many of each component's product terms go to GPSIMD.
    """
    aw, ax, ay, az = a
    bw, bx, by, bz = b
    if sparse_z:
        comp_terms = [
            (out[0], [(1, aw, bw), (-1, ax, bx), (-1, ay, by)]),
            (out[1], [(1, aw, bx), (1, ax, bw)]),
            (out[2], [(1, aw, by), (1, ay, bw)]),
            (out[3], [(1, ax, by), (-1, ay, bx)]),
        ]
    else:
        comp_terms = [
            (out[0], [(1, aw, bw), (-1, ax, bx), (-1, ay, by), (-1, az, bz)]),
            (out[1], [(1, aw, bx), (1, ax, bw), (1, ay, bz), (-1, az, by)]),
            (out[2], [(1, aw, by), (1, ay, bw), (1, az, bx), (-1, ax, bz)]),
            (out[3], [(1, aw, bz), (1, ax, by), (1, az, bw), (-1, ay, bx)]),
        ]
    for dst, terms in comp_terms:
        tmps = []
        for ti, (sign, u, v) in enumerate(terms):
            if ti == 0:
                nc.vector.tensor_mul(dst, u, v)
            else:
                t = pool.tile([128, NSCAN], F32, tag=f"qt{ti}", name=f"qt{ti}")
                eng = nc.gpsimd if ti <= gpsimd_mults else nc.vector
                eng.tensor_mul(t[:, :width], u, v)
                tmps.append((sign, t))
        for sign, t in tmps:
            op = Alu.add if sign > 0 else Alu.subtract
            nc.vector.tensor_tensor(dst, dst, t[:, :width], op)


def emit_backmap(ctx: ExitStack, tc: tile.TileContext, out_ap, dist_ap, ang_ap, dih_ap):
    nc = tc.nc
    pool = ctx.enter_context(tc.tile_pool(name="main", bufs=1))
    tpool = ctx.enter_context(tc.tile_pool(name="tmp", bufs=2))
    ppool = ctx.enter_context(tc.tile_pool(name="psum", bufs=1, space="PSUM"))

    # ---------------- Phase A: lengths = mean over full batch ----------------
    dts = []
    for i in range(4):
        dt = pool.tile([128, N_DIST], F32, tag=f"dist{i}", name=f"dist{i}")
        nc.sync.dma_start(dt[:], dist_ap[128 * i:128 * (i + 1), :])
        dts.append(dt)
    ones_col = pool.tile([128, 1], F32)
    nc.vector.memset(ones_col[:], 1.0)
    psl = ppool.tile([1, N_DIST], F32)
    for i in range(4):
        nc.tensor.matmul(psl[:], ones_col[:], dts[i][:],
                         start=(i == 0), stop=(i == 3))
    sb_len = pool.tile([1, N_DIST], F32)
    nc.scalar.mul(sb_len[:], psl[:], 1.0 / B_FULL)
    ones_row = pool.tile([1, 128], F32)
    nc.vector.memset(ones_row[:], 1.0)
    psb = ppool.tile([128, N_DIST], F32)
    nc.tensor.matmul(psb[:], ones_row[:], sb_len[:], start=True, stop=True)
    lenb = pool.tile([128, N_DIST], F32)
    nc.vector.tensor_copy(lenb[:], psb[:])

    # ---------------- Phase B: shared in-plane chain ----------------
    angt = pool.tile([128, N_ANG], F32)
    diht = pool.tile([128, N_DIH], F32)
    for rows in (R, L):
        nc.sync.dma_start(angt[rows, :], ang_ap[:, :])
        nc.sync.dma_start(diht[rows, :], dih_ap[:, :])

    ones = pool.tile([128, 512], F32)
    nc.vector.memset(ones[:], 1.0)

    bz = pool.tile([128, 1], F32)
    bpi2 = pool.tile([128, 1], F32)
    nc.vector.memset(bz[:], 0.0)
    nc.vector.memset(bpi2[:], PI / 2.0)

    # integrand = sa * (pi - ang) = (ang * -1 + pi) * sa
    integ = pool.tile([128, N_ANG], F32)
    nc.vector.tensor_scalar(integ[:], angt[:], -1.0, PI, Alu.mult, Alu.add)
    nc.vector.tensor_scalar_mul(integ[:, 1:N_ANG:2], integ[:, 1:N_ANG:2], -1.0)

    # theta[0]=0, theta[1:511] = sa * cumsum(integrand)
    th = pool.tile([128, N_DIST], F32)
    nc.vector.tensor_tensor_scan(th[:, 1:N_DIST], ones[:, 0:N_ANG], integ[:],
                                 0.0, Alu.mult, Alu.add)
    nc.vector.memset(th[:, 0:1], 0.0)
    nc.vector.tensor_scalar_mul(th[:, 2:N_DIST:2], th[:, 2:N_DIST:2], -1.0)

    # range-reduce theta (Cody-Waite: k = int(theta/2pi), thm = theta - k*2pi)
    kq = pool.tile([128, N_DIST], F32)
    ki = pool.tile([128, N_DIST], mybir.dt.int32)
    kf = pool.tile([128, N_DIST], F32)
    nc.vector.tensor_scalar_mul(kq[:], th[:], 1.0 / TWO_PI)
    nc.vector.tensor_copy(ki[:], kq[:])
    nc.vector.tensor_copy(kf[:], ki[:])
    thm = pool.tile([128, N_DIST], F32)
    nc.vector.cody_waite_cascade(thm[:], th[:], kf[:],
                                 6.28125, 0.0019350051879882812,
                                 3.019916050561733e-07)
    sw = pool.tile([128, N_DIST], F32)
    cw = pool.tile([128, N_DIST], F32)
    nc.vector.add_range_wrap(sw[:], thm[:], 0.0, PI, TWO_PI)
    nc.vector.add_range_wrap(cw[:], thm[:], PI / 2.0, PI, TWO_PI)
    sinq = pool.tile([128, N_DIST], F32)   # becomes uy (signed sin)
    cosq = pool.tile([128, N_DIST], F32)   # ux
    nc.scalar.activation(sinq[:], sw[:], Act.Sin, bias=bz[:, :])
    nc.scalar.activation(cosq[:], cw[:], Act.Sin, bias=bz[:, :])
    nc.vector.tensor_scalar_mul(sinq[:, 1:N_DIST:2], sinq[:, 1:N_DIST:2], -1.0)

    dxt = pool.tile([128, N_DIST], F32)
    dyt = pool.tile([128, N_DIST], F32)
    nc.vector.tensor_mul(dxt[:], lenb[:], cosq[:])
    nc.vector.tensor_mul(dyt[:], lenb[:], sinq[:])

    xs = pool.tile([128, N_ATOMS], F32)
    ys = pool.tile([128, N_ATOMS], F32)
    nc.vector.memset(xs[:, 0:1], 0.0)
    nc.vector.memset(ys[:, 0:1], 0.0)
    nc.vector.tensor_tensor_scan(xs[:, 1:N_ATOMS], ones[:, 0:N_DIST], dxt[:],
                                 0.0, Alu.mult, Alu.add)
    nc.vector.tensor_tensor_scan(ys[:, 1:N_ATOMS], ones[:, 0:N_DIST], dyt[:],
                                 0.0, Alu.mult, Alu.add)

    # half-angle trig of dih+pi:  C = cos((dih+pi)/2) = Sin(-dih/2),
    #                             S = sin((dih+pi)/2) = Sin(dih/2 + pi/2)
    Ct = pool.tile([128, N_DIH], F32)
    St = pool.tile([128, N_DIH], F32)
    nc.scalar.activation(Ct[:], diht[:], Act.Sin, bias=bz[:, :], scale=-0.5)
    nc.scalar.activation(St[:], diht[:], Act.Sin, bias=bpi2[:, :], scale=0.5)

    # ---------------- Phase C: chain tiles [128, 256] ----------------
    # position k pairs bond j=k+1 with prefix quat h_k.
    gw = pool.tile([128, NSCAN], F32)
    gx = pool.tile([128, NSCAN], F32)
    gy = pool.tile([128, NSCAN], F32)
    gz = pool.tile([128, NSCAN], F32)
    sfac = pool.tile([128, NSCAN], F32)
    axt = pool.tile([128, NSCAN], F32)
    ayt = pool.tile([128, NSCAN], F32)
    bx = pool.tile([128, NSCAN], F32)
    by = pool.tile([128, NSCAN], F32)

    # gw = C slices; right: C[255+k], k<254; left: C[254-k], k<255
    nc.scalar.copy(gw[R, 0:254], Ct[R, 255:509])
    nc.scalar.copy(gw[L, 0:255], Ct[L, 254::-1])
    nc.vector.memset(gw[R, 254:256], 1.0)
    nc.vector.memset(gw[L, 255:256], 1.0)
    # sfac: right +S[255+k]; left -S[254-k]
    nc.scalar.copy(sfac[R, 0:254], St[R, 255:509])
    nc.vector.tensor_scalar_mul(sfac[L, 0:255], St[L, 254::-1], -1.0)
    nc.vector.memset(sfac[R, 254:256], 0.0)
    nc.vector.memset(sfac[L, 255:256], 0.0)
    # axis slices: right u[256+k]; left u[255-k]
    nc.scalar.copy(axt[R, 0:254], cosq[R, 256:510])
    nc.scalar.copy(axt[L, 0:255], cosq[L, 255:0:-1])
    nc.vector.memset(axt[R, 254:256], 0.0)
    nc.vector.memset(axt[L, 255:256], 0.0)
    nc.scalar.copy(ayt[R, 0:254], sinq[R, 256:510])
    nc.scalar.copy(ayt[L, 0:255], sinq[L, 255:0:-1])
    nc.vector.memset(ayt[R, 254:256], 0.0)
    nc.vector.memset(ayt[L, 255:256], 0.0)
    nc.vector.tensor_mul(gx[:], sfac[:], axt[:])
    nc.vector.tensor_mul(gy[:], sfac[:], ayt[:])
    nc.vector.memset(gz[:], 0.0)
    # bonds at position k: right dx[257+k] (k<254); left -dx[254-k] (k<255)
    nc.scalar.copy(bx[R, 0:254], dxt[R, 257:511])
    nc.vector.tensor_scalar_mul(bx[L, 0:255], dxt[L, 254::-1], -1.0)
    nc.vector.memset(bx[R, 254:256], 0.0)
    nc.vector.memset(bx[L, 255:256], 0.0)
    nc.scalar.copy(by[R, 0:254], dyt[R, 257:511])
    nc.vector.tensor_scalar_mul(by[L, 0:255], dyt[L, 254::-1], -1.0)
    nc.vector.memset(by[R, 254:256], 0.0)
    nc.vector.memset(by[L, 255:256], 0.0)

    # ---------------- Phase D: Hillis-Steele quaternion prefix scan ----------
    setA = tuple(pool.tile([128, NSCAN], F32, tag=f"qa{i}", name=f"qa{i}") for i in range(4))
    setB = tuple(pool.tile([128, NSCAN], F32, tag=f"qb{i}", name=f"qb{i}") for i in range(4))
    cur = (gw, gx, gy, gz)
    nxt_sets = [setA, setB]
    for si, s in enumerate([1, 2, 4, 8, 16, 32, 64, 128]):
        nxt = nxt_sets[si % 2]
        w = NSCAN - s
        a = tuple(c[:, 0:w] for c in cur)          # h[k-s]
        b = tuple(c[:, s:NSCAN] for c in cur)      # h[k]
        o = tuple(c[:, s:NSCAN] for c in nxt)
        for ci in range(4):
            nc.scalar.copy(nxt[ci][:, 0:s], cur[ci][:, 0:s])
        _emit_quat_mul(nc, tpool, o, a, b, w, sparse_z=(si == 0))
        cur = nxt

    hw_, hx, hy, hz = cur

    # ---------------- Phase E: rotate bonds by h ----------------
    def tl():
        return tpool.tile([128, NSCAN], F32, tag="rot", name="rot")

    p1 = pool.tile([128, NSCAN], F32)
    p2 = pool.tile([128, NSCAN], F32)
    tz = pool.tile([128, NSCAN], F32)
    cxt = pool.tile([128, NSCAN], F32)
    cyt = pool.tile([128, NSCAN], F32)
    czt = pool.tile([128, NSCAN], F32)
    rx = pool.tile([128, NSCAN], F32)
    ry = pool.tile([128, NSCAN], F32)
    rz = pool.tile([128, NSCAN], F32)

    nc.vector.tensor_mul(p1[:], hz[:], by[:])
    nc.vector.tensor_mul(p2[:], hz[:], bx[:])
    m3 = tl(); nc.gpsimd.tensor_mul(m3[:], hx[:], by[:])
    m4 = tl(); nc.gpsimd.tensor_mul(m4[:], hy[:], bx[:])
    nc.vector.tensor_tensor(tz[:], m3[:], m4[:], Alu.subtract)
    m5 = tl(); nc.vector.tensor_mul(m5[:], hy[:], tz[:])
    m6 = tl(); nc.gpsimd.tensor_mul(m6[:], hz[:], p2[:])
    nc.vector.tensor_tensor(cxt[:], m5[:], m6[:], Alu.subtract)
    m7 = tl(); nc.gpsimd.tensor_mul(m7[:], hz[:], p1[:])
    m8 = tl(); nc.vector.tensor_mul(m8[:], hx[:], tz[:])
    nc.vector.tensor_tensor(cyt[:], m7[:], m8[:], Alu.add)
    m9 = tl(); nc.gpsimd.tensor_mul(m9[:], hx[:], p2[:])
    m10 = tl(); nc.vector.tensor_mul(m10[:], hy[:], p1[:])
    nc.vector.tensor_tensor(czt[:], m9[:], m10[:], Alu.add)
    m11 = tl(); nc.vector.tensor_mul(m11[:], hw_[:], p1[:])
    s1 = tl(); nc.vector.tensor_tensor(s1[:], cxt[:], m11[:], Alu.subtract)
    m12 = tl(); nc.vector.tensor_mul(m12[:], hw_[:], p2[:])
    s2 = tl(); nc.vector.tensor_tensor(s2[:], m12[:], cyt[:], Alu.subtract)
    m13 = tl(); nc.gpsimd.tensor_mul(m13[:], hw_[:], tz[:])
    s3 = tl(); nc.vector.tensor_tensor(s3[:], m13[:], czt[:], Alu.add)
    nc.vector.scalar_tensor_tensor(rx[:], s1[:], 2.0, bx[:], Alu.mult, Alu.add)
    nc.vector.scalar_tensor_tensor(ry[:], s2[:], 2.0, by[:], Alu.mult, Alu.add)
    nc.vector.tensor_scalar_mul(rz[:], s3[:], 2.0)

    # ---------------- Phase F: anchors + position scans ----------------
    p1x = pool.tile([128, 1], F32)
    p1y = pool.tile([128, 1], F32)
    nc.scalar.copy(p1x[R, :], xs[R, 257:258])
    nc.scalar.copy(p1x[L, :], xs[L, 255:256])
    nc.scalar.copy(p1y[R, :], ys[R, 257:258])
    nc.scalar.copy(p1y[L, :], ys[L, 255:256])

    asm_r = pool.tile([128, 762], F32)
    asm_l = pool.tile([128, 765], F32)
    # right: scan value at k = P_{k+2} = out atom 258+k, k=0..253
    nc.vector.tensor_tensor_scan(asm_r[R, 0:762:3], ones[R, 0:254], rx[R, 0:254],
                                 p1x[R, :], Alu.mult, Alu.add)
    nc.vector.tensor_tensor_scan(asm_r[R, 1:762:3], ones[R, 0:254], ry[R, 0:254],
                                 p1y[R, :], Alu.mult, Alu.add)
    nc.vector.tensor_tensor_scan(asm_r[R, 2:762:3], ones[R, 0:254], rz[R, 0:254],
                                 0.0, Alu.mult, Alu.add)
    # left: scan value at k = P_{k+2} = out atom 254-k, k=0..254 (reversed cols)
    nc.vector.tensor_tensor_scan(asm_l[L, 762::-3], ones[L, 0:255], rx[L, 0:255],
                                 p1x[L, :], Alu.mult, Alu.add)
    nc.vector.tensor_tensor_scan(asm_l[L, 763:0:-3], ones[L, 0:255], ry[L, 0:255],
                                 p1y[L, :], Alu.mult, Alu.add)
    nc.vector.tensor_tensor_scan(asm_l[L, 764:1:-3], ones[L, 0:255], rz[L, 0:255],
                                 0.0, Alu.mult, Alu.add)

    mid = pool.tile([128, 9], F32)
    nc.scalar.copy(mid[R, 0:7:3], xs[R, 255:258])
    nc.scalar.copy(mid[R, 1:8:3], ys[R, 255:258])
    nc.vector.memset(mid[R, 2:9:3], 0.0)

    # ---------------- Phase G: output DMAs ----------------
    nc.sync.dma_start(out_ap[:, 258:512, :],
                      asm_r[R, 0:762].rearrange("p (a c) -> p a c", c=3))
    nc.sync.dma_start(out_ap[:, 0:255, :],
                      asm_l[L, 0:765].rearrange("p (a c) -> p a c", c=3))
    nc.sync.dma_start(out_ap[:, 255:258, :],
                      mid[R, 0:9].rearrange("p (a c) -> p a c", c=3))


def build():
    nc = bacc.Bacc("TRN2", target_bir_lowering=False, debug=False)
    dist_t = nc.dram_tensor("dist", [B_FULL, N_DIST], F32, kind="ExternalInput")
    ang_t = nc.dram_tensor("ang", [NB, N_ANG], F32, kind="ExternalInput")
    dih_t = nc.dram_tensor("dih", [NB, N_DIH], F32, kind="ExternalInput")
    out_t = nc.dram_tensor("out", [NB, N_ATOMS, 3], F32, kind="ExternalOutput")
    with tile.TileContext(nc) as tc:
        with ExitStack() as ctx:
            emit_backmap(ctx, tc, out_t.ap(), dist_t.ap(), ang_t.ap(), dih_t.ap())
    nc.compile()
    return nc


_NC_CACHE = None


def kernel(distances, angles, dihedrals, left_split=255, right_split=254):
    global _NC_CACHE
    assert int(left_split) == LEFT_SPLIT and int(right_split) == RIGHT_SPLIT
    distances = np.ascontiguousarray(distances, dtype=np.float32)
    angles = np.ascontiguousarray(angles, dtype=np.float32)
    dihedrals = np.ascontiguousarray(dihedrals, dtype=np.float32)
    assert distances.shape == (B_FULL, N_DIST)
    if _NC_CACHE is None:
        _NC_CACHE = build()
    nc = _NC_CACHE
    in_maps = []
    for c in range(N_CORES):
        rows = slice(NB * c, NB * (c + 1))
        in_maps.append({
            "dist": distances,
            "ang": angles[rows],
            "dih": dihedrals[rows],
        })
    res = bass_utils.run_bass_kernel_spmd(
        nc, in_maps, core_ids=list(range(N_CORES)),
        trace=bool(int(os.environ.get("BK_TRACE", "0"))),
    )
    out = np.concatenate([r["out"] for r in res.results], axis=0)
    if res.exec_time_ns is not None:
        kernel.last_exec_time_ns = res.exec_time_ns
    return out


kernel.last_exec_time_ns = None
